# revision 21
# baseline (speedup 1.0000x reference)
"""BitLinear (int8-activation x ternary-weight) matmul on 8 TRN2 NeuronCores.

Full inputs: x [4, 4096, 2048] f32, weight [2048, 2048] f32.
Output: [4, 4096, 2048] fp16.

Strategy: data-parallel over the 16384 rows (2048 rows/core); weight
replicated and quantized on-device to ternary fp8 (two reads of W; the
first NCACHE k-tiles stay resident in SBUF so only the rest are re-read).
Activations are quantized per-row to int8, then split qx = h + l with
h = round16(x*si) (multiple of 16, |h|<=128) and l = qx - h (|l|<=8) --
both exactly representable in fp8e4. The matmul runs in fp8 DoubleRow
mode (2 MACs/cell/cycle): stationary holds (h,l) pairs, the moving
operand replays each ternary qw value twice via a 0-step AP dimension,
so out = sum_k (h+l)*qw = sum_k qx*qw exactly (fp32 PSUM accumulation).
Host only reshapes/shards and transposes W (layout prep, no math).
"""

import numpy as np

import concourse.bass as bass
import concourse.mybir as mybir
import concourse.tile as tile
from concourse import bacc
from concourse.bass import ts
from concourse.bass_utils import run_bass_kernel_spmd

N_CORES = 8
ROWS_TOTAL = 4 * 4096
K = 2048
N = 2048
NCACHE = 7  # W k-tiles kept in SBUF between the mean pass and quantize pass
MAGIC = 12582912.0  # 1.5*2^23: fp32 round-to-nearest-even (both signs)
MAGIC16 = 201326592.0  # 1.5*2^27: fp32 round-to-nearest-multiple-of-16

f32 = mybir.dt.float32
bf16 = mybir.dt.bfloat16
f16 = mybir.dt.float16
fp8 = mybir.dt.float8e4
Alu = mybir.AluOpType
Act = mybir.ActivationFunctionType
AxX = mybir.AxisListType.X
DR = mybir.MatmulPerfMode.DoubleRow


def build(rows_per_core=ROWS_TOTAL // N_CORES):
    nc = bacc.Bacc(
        "TRN2", target_bir_lowering=False, debug=False, num_devices=N_CORES
    )
    x_ext = nc.declare_dram_parameter("x", [rows_per_core, K], f32, isOutput=False)
    wt_ext = nc.declare_dram_parameter("wt", [K, N], f32, isOutput=False)
    out_ext = nc.declare_dram_parameter(
        "out", [rows_per_core, N], f16, isOutput=True
    )

    KT = K // 128
    MT = rows_per_core // 128
    NQ = N // 512
    NPRE = min(4, MT)  # x tiles prefetched during W prep

    with tile.TileContext(nc) as tc:
        with (
            tc.tile_pool(name="xin", bufs=4) as xin,  # [128,K] f32 x loads
            tc.tile_pool(name="wld", bufs=3) as wld,  # [128,K] f32 W loads
            tc.tile_pool(name="wch", bufs=NCACHE) as wch,  # cached W tiles
            tc.tile_pool(name="scaled", bufs=2) as scaled,  # [128,K] f32 ACT out
            tc.tile_pool(name="qtmp", bufs=2) as qtmp,  # rounded f32 / qx bf16
            tc.tile_pool(name="qxt", bufs=2) as qxtp,  # [128,KT,128] bf16 x^T
            tc.tile_pool(name="outp", bufs=3) as outp,  # [128,N] f16 results
            tc.tile_pool(name="singles", bufs=1) as singles,
            tc.tile_pool(name="small", bufs=6) as small,  # [128,1] stats
            tc.tile_pool(name="pacc", bufs=7, space="PSUM") as pacc,
            tc.tile_pool(name="pt", bufs=1, space="PSUM") as pt,
        ):
            ones_col = singles.tile([128, 1], f32)
            nc.vector.memset(ones_col, 1.0)
            ones_row = singles.tile([1, 128], f32)
            nc.vector.memset(ones_row, 1.0)
            qwT = singles.tile([128, KT, N], fp8)
            negmagic = singles.tile([128, 1], f32)
            nc.vector.memset(negmagic, -MAGIC)
            wsums = singles.tile([128, KT], f32)

            def x_quant(mi):
                if mi in x_pre:
                    x_t = x_pre[mi]
                else:
                    x_t = xin.tile([128, K], f32, tag="xin", name=f"x{mi}")
                    nc.sync.dma_start(out=x_t, in_=x_ext[ts(mi, 128), :])
                amax = small.tile([128, 1], f32, tag="small")
                nc.vector.tensor_reduce(
                    out=amax, in_=x_t, axis=AxX, op=Alu.max,
                    apply_absolute_value=True,
                )
                amc = small.tile([128, 1], f32, tag="amc", name=f"amc{mi}")
                nc.vector.tensor_scalar_max(out=amc, in0=amax, scalar1=1e-5)
                rec = small.tile([128, 1], f32, tag="small")
                nc.vector.reciprocal(out=rec, in_=amc)
                si = small.tile([128, 1], f32, tag="small")
                nc.vector.tensor_scalar_mul(out=si, in0=rec, scalar1=127.0)
                xs = scaled.tile([128, K], f32, tag="scaled")
                nc.scalar.activation(out=xs, in_=x_t, func=Act.Copy, scale=si)
                qx = qtmp.tile([128, K], bf16, tag="qtmp")
                nc.vector.tensor_scalar(
                    out=qx, in0=xs, scalar1=MAGIC, scalar2=-MAGIC,
                    op0=Alu.add, op1=Alu.add,
                )
                qxT = qxtp.tile(
                    [128, KT, 128], bf16, tag="qxt", name=f"qxT{mi}"
                )
                nc.scalar.dma_start_transpose(out=qxT, in_=qx)
                return qxT, amc

            # ---- W pass 1: mean(|W|); cache k-tiles 0..NCACHE-1 in SBUF.
            # x prefetch DMAs are interleaved into the W read stream (same
            # HWDGE FIFO) so the first row tiles land early enough to be
            # quantized while the W mean is still accumulating.
            wcache_tiles = {}
            x_pre = {}
            npre_done = 0
            for kt in range(KT):
                if kt < NCACHE:
                    wt_t = wch.tile([128, K], f32, tag="wch", name=f"wch{kt}")
                    wcache_tiles[kt] = wt_t
                else:
                    wt_t = wld.tile([128, K], f32, tag="wld", name=f"wld{kt}")
                nc.sync.dma_start(out=wt_t, in_=wt_ext[ts(kt, 128), :])
                nc.vector.tensor_reduce(
                    out=wsums[:, kt : kt + 1],
                    in_=wt_t,
                    axis=AxX,
                    op=Alu.add,
                    apply_absolute_value=True,
                )
                if kt % 4 == 3 and npre_done < NPRE:
                    mi = npre_done
                    x_t = xin.tile([128, K], f32, tag="xin", name=f"xpre{mi}")
                    nc.sync.dma_start(out=x_t, in_=x_ext[ts(mi, 128), :])
                    x_pre[mi] = x_t
                    npre_done += 1

            xq0 = x_quant(0)
            xq1 = x_quant(1) if MT >= 2 else None
            # re-read DMAs for non-cached tiles (no dependency on sw)
            wreread_tiles = {}
            for kt in range(NCACHE, KT):
                wt_t = wld.tile([128, K], f32, tag="wld", name=f"wldr{kt}")
                nc.sync.dma_start(out=wt_t, in_=wt_ext[ts(kt, 128), :])
                wreread_tiles[kt] = wt_t
            wtot = small.tile([128, 1], f32, tag="small")
            nc.vector.tensor_reduce(out=wtot, in_=wsums, axis=AxX, op=Alu.add)
            ptot = pt.tile([1, 1], f32, tag="pt")
            nc.tensor.matmul(ptot, lhsT=ones_col, rhs=wtot, start=True, stop=True)
            # meanc = max(mean|W|, 1e-5); sw = 1/meanc; q = meanc/127
            s_meanc = small.tile([1, 1], f32, tag="s1")
            nc.vector.tensor_scalar(
                out=s_meanc,
                in0=ptot,
                scalar1=1.0 / (K * N),
                scalar2=1e-5,
                op0=Alu.mult,
                op1=Alu.max,
            )
            s_sw = small.tile([1, 1], f32, tag="s1")
            nc.vector.reciprocal(out=s_sw, in_=s_meanc)
            s_q = small.tile([1, 1], f32, tag="s1")
            nc.vector.tensor_scalar_mul(out=s_q, in0=s_meanc, scalar1=1.0 / 127.0)
            # broadcast scalars to all 128 partitions via PE outer product
            pb = pt.tile([128, 1], f32, tag="pt")
            nc.tensor.matmul(pb, lhsT=ones_row, rhs=s_sw, start=True, stop=True)
            sw_b = singles.tile([128, 1], f32)
            nc.vector.tensor_copy(out=sw_b, in_=pb)
            pb2 = pt.tile([128, 1], f32, tag="pt")
            nc.tensor.matmul(pb2, lhsT=ones_row, rhs=s_q, start=True, stop=True)
            q_b = singles.tile([128, 1], f32)
            nc.vector.tensor_copy(out=q_b, in_=pb2)

            # ---- W pass 2: qwT = clip(round(wT*sw), -1, 1) as fp8
            # Two passes, one per engine: DVE computes u = w*sw + MAGIC
            # in-place (the fp32 add rounds to the nearest integer, RNE);
            # ACT then emits Sign(u - MAGIC) straight to fp8 -- for integer
            # n, sign(n) == clip(n, -1, 1).
            for kt in range(KT):
                wt_t = wcache_tiles.get(kt) or wreread_tiles.get(kt)
                nc.vector.tensor_scalar(
                    out=wt_t, in0=wt_t, scalar1=sw_b, scalar2=MAGIC,
                    op0=Alu.mult, op1=Alu.add,
                )
                nc.scalar.activation(
                    out=qwT[:, kt, :], in_=wt_t, func=Act.Sign, bias=negmagic
                )

            # ---- main loop over row tiles
            def mm(acc, qxT, kt, nq):
                nc.tensor.matmul(
                    acc, lhsT=qxT[:, kt, :], rhs=qwT[:, kt, ts(nq, 512)],
                    start=(kt == 0), stop=(kt == KT - 1),
                    skip_group_check=True,
                )

            def finish(mi, accs, amc):
                cs = small.tile([128, 1], f32, tag="small")
                nc.vector.tensor_mul(cs, amc, q_b)  # (amax/127)*meanc
                o_t = outp.tile([128, N], f16, tag="outp", name=f"o{mi}")
                for nq in range(NQ):
                    nc.scalar.activation(
                        out=o_t[:, ts(nq, 512)], in_=accs[nq],
                        func=Act.Copy, scale=cs,
                    )
                nc.scalar.dma_start(out=out_ext[ts(mi, 128), :], in_=o_t)

            if MT >= 2:
                # interleave the first two row tiles across kt so each
                # arriving qwT k-tile unlocks 7 matmuls during the W-prep ramp
                qxT0, amc0 = xq0
                qxT1, amc1 = xq1
                accs0 = [
                    pacc.tile([128, 512], f32, tag="acc", name=f"acc_0_{i}")
                    for i in range(NQ)
                ]
                accs1 = [
                    pacc.tile([128, 512], f32, tag="acc", name=f"acc_1_{i}")
                    for i in range(NQ - 1)
                ]
                for kt in range(KT):
                    for nq in range(NQ):
                        mm(accs0[nq], qxT0, kt, nq)
                    for nq in range(NQ - 1):
                        mm(accs1[nq], qxT1, kt, nq)
                finish(0, accs0, amc0)
                acc1_last = pacc.tile(
                    [128, 512], f32, tag="acc", name="acc_1_3"
                )
                for kt in range(KT):
                    mm(acc1_last, qxT1, kt, NQ - 1)
                finish(1, accs1 + [acc1_last], amc1)
                start_mi = 2
            else:
                start_mi = 0

            for mi in range(start_mi, MT):
                qxT, amc = x_quant(mi)
                accs = [
                    pacc.tile([128, 512], f32, tag="acc", name=f"acc_{mi}_{i}")
                    for i in range(NQ)
                ]
                for kt in range(KT):
                    for nq in range(NQ):
                        mm(accs[nq], qxT, kt, nq)
                finish(mi, accs, amc)

    nc.compile()
    return nc


_NC_CACHE = {}


def _get_nc(rows_per_core):
    if rows_per_core not in _NC_CACHE:
        _NC_CACHE[rows_per_core] = build(rows_per_core)
    return _NC_CACHE[rows_per_core]


def run(x, weight, **spmd_kwargs):
    x = np.ascontiguousarray(np.asarray(x, dtype=np.float32))
    weight = np.asarray(weight, dtype=np.float32)
    b, s, k = x.shape
    rows = b * s
    rpc = rows // N_CORES
    xr = x.reshape(rows, k)
    wt = np.ascontiguousarray(weight.T)
    nc = _get_nc(rpc)
    in_maps = [
        {"x": xr[i * rpc : (i + 1) * rpc], "wt": wt} for i in range(N_CORES)
    ]
    res = run_bass_kernel_spmd(
        nc, in_maps, core_ids=list(range(N_CORES)), **spmd_kwargs
    )
    out = np.concatenate(
        [res.results[i]["out"] for i in range(N_CORES)], axis=0
    )
    return out.reshape(b, s, N), res


def kernel(x, weight):
    out, _ = run(x, weight)
    return out


# revision 22
# speedup vs baseline: 1.0049x; 1.0049x over previous
"""BitLinear (int8-activation x ternary-weight) matmul on 8 TRN2 NeuronCores.

Full inputs: x [4, 4096, 2048] f32, weight [2048, 2048] f32.
Output: [4, 4096, 2048] fp16.

Strategy: data-parallel over the 16384 rows (2048 rows/core); weight
replicated and quantized on-device to ternary fp8 (two reads of W; the
first NCACHE k-tiles stay resident in SBUF so only the rest are re-read).
Activations are quantized per-row to int8, then split qx = h + l with
h = round16(x*si) (multiple of 16, |h|<=128) and l = qx - h (|l|<=8) --
both exactly representable in fp8e4. The matmul runs in fp8 DoubleRow
mode (2 MACs/cell/cycle): stationary holds (h,l) pairs, the moving
operand replays each ternary qw value twice via a 0-step AP dimension,
so out = sum_k (h+l)*qw = sum_k qx*qw exactly (fp32 PSUM accumulation).
Host only reshapes/shards and transposes W (layout prep, no math).
"""

import numpy as np

import concourse.bass as bass
import concourse.mybir as mybir
import concourse.tile as tile
from concourse import bacc
from concourse.bass import ts
from concourse.bass_utils import run_bass_kernel_spmd

N_CORES = 8
ROWS_TOTAL = 4 * 4096
K = 2048
N = 2048
NCACHE = 7  # W k-tiles kept in SBUF between the mean pass and quantize pass
MAGIC = 12582912.0  # 1.5*2^23: fp32 round-to-nearest-even (both signs)
MAGIC16 = 201326592.0  # 1.5*2^27: fp32 round-to-nearest-multiple-of-16

f32 = mybir.dt.float32
bf16 = mybir.dt.bfloat16
f16 = mybir.dt.float16
fp8 = mybir.dt.float8e4
Alu = mybir.AluOpType
Act = mybir.ActivationFunctionType
AxX = mybir.AxisListType.X
DR = mybir.MatmulPerfMode.DoubleRow


def build(rows_per_core=ROWS_TOTAL // N_CORES):
    nc = bacc.Bacc(
        "TRN2", target_bir_lowering=False, debug=False, num_devices=N_CORES
    )
    x_ext = nc.declare_dram_parameter("x", [rows_per_core, K], f32, isOutput=False)
    wt_ext = nc.declare_dram_parameter("wt", [K, N], f32, isOutput=False)
    out_ext = nc.declare_dram_parameter(
        "out", [rows_per_core, N], f16, isOutput=True
    )

    KT = K // 128
    MT = rows_per_core // 128
    NQ = N // 512
    NPRE = min(4, MT)  # x tiles prefetched during W prep

    with tile.TileContext(nc) as tc:
        with (
            tc.tile_pool(name="xin", bufs=4) as xin,  # [128,K] f32 x loads
            tc.tile_pool(name="wld", bufs=3) as wld,  # [128,K] f32 W loads
            tc.tile_pool(name="wch", bufs=NCACHE) as wch,  # cached W tiles
            tc.tile_pool(name="scaled", bufs=2) as scaled,  # [128,K] f32 ACT out
            tc.tile_pool(name="qtmp", bufs=2) as qtmp,  # rounded f32 / qx bf16
            tc.tile_pool(name="qxt", bufs=2) as qxtp,  # [128,KT,128] bf16 x^T
            tc.tile_pool(name="outp", bufs=3) as outp,  # [128,N] f16 results
            tc.tile_pool(name="singles", bufs=1) as singles,
            tc.tile_pool(name="small", bufs=6) as small,  # [128,1] stats
            tc.tile_pool(name="pacc", bufs=7, space="PSUM") as pacc,
            tc.tile_pool(name="pt", bufs=1, space="PSUM") as pt,
        ):
            ones_col = singles.tile([128, 1], f32)
            nc.vector.memset(ones_col, 1.0)
            ones_row = singles.tile([1, 128], f32)
            nc.vector.memset(ones_row, 1.0)
            qwT = singles.tile([128, KT, N], fp8)
            negmagic = singles.tile([128, 1], f32)
            nc.vector.memset(negmagic, -MAGIC)
            wsums = singles.tile([128, KT], f32)

            def x_quant(mi):
                if mi in x_pre:
                    x_t = x_pre[mi]
                else:
                    x_t = xin.tile([128, K], f32, tag="xin", name=f"x{mi}")
                    nc.sync.dma_start(out=x_t, in_=x_ext[ts(mi, 128), :])
                amax = small.tile([128, 1], f32, tag="small")
                nc.vector.tensor_reduce(
                    out=amax, in_=x_t, axis=AxX, op=Alu.max,
                    apply_absolute_value=True,
                )
                amc = small.tile([128, 1], f32, tag="amc", name=f"amc{mi}")
                nc.vector.tensor_scalar_max(out=amc, in0=amax, scalar1=1e-5)
                rec = small.tile([128, 1], f32, tag="small")
                nc.vector.reciprocal(out=rec, in_=amc)
                si = small.tile([128, 1], f32, tag="small")
                nc.vector.tensor_scalar_mul(out=si, in0=rec, scalar1=127.0)
                xs = scaled.tile([128, K], f32, tag="scaled")
                nc.scalar.activation(out=xs, in_=x_t, func=Act.Copy, scale=si)
                qx = qtmp.tile([128, K], bf16, tag="qtmp")
                nc.vector.tensor_scalar(
                    out=qx, in0=xs, scalar1=MAGIC, scalar2=-MAGIC,
                    op0=Alu.add, op1=Alu.add,
                )
                qxT = qxtp.tile(
                    [128, KT, 128], bf16, tag="qxt", name=f"qxT{mi}"
                )
                nc.scalar.dma_start_transpose(out=qxT, in_=qx)
                return qxT, amc

            # ---- W pass 1: mean(|W|); cache k-tiles 0..NCACHE-1 in SBUF
            wcache_tiles = {}
            for kt in range(KT):
                if kt < NCACHE:
                    wt_t = wch.tile([128, K], f32, tag="wch", name=f"wch{kt}")
                    wcache_tiles[kt] = wt_t
                else:
                    wt_t = wld.tile([128, K], f32, tag="wld", name=f"wld{kt}")
                nc.sync.dma_start(out=wt_t, in_=wt_ext[ts(kt, 128), :])
                nc.vector.tensor_reduce(
                    out=wsums[:, kt : kt + 1],
                    in_=wt_t,
                    axis=AxX,
                    op=Alu.add,
                    apply_absolute_value=True,
                )
            # first two x tiles next in the input FIFO: needed for the ramp
            x_pre = {}
            for mi in range(min(2, MT)):
                x_t = xin.tile([128, K], f32, tag="xin", name=f"xpre{mi}")
                nc.sync.dma_start(out=x_t, in_=x_ext[ts(mi, 128), :])
                x_pre[mi] = x_t
            xq0 = x_quant(0)
            xq1 = x_quant(1) if MT >= 2 else None
            # re-read DMAs for non-cached tiles (no dependency on sw)
            wreread_tiles = {}
            for kt in range(NCACHE, KT):
                wt_t = wld.tile([128, K], f32, tag="wld", name=f"wldr{kt}")
                nc.sync.dma_start(out=wt_t, in_=wt_ext[ts(kt, 128), :])
                wreread_tiles[kt] = wt_t
            # then the next prefetched x tiles
            for mi in range(2, NPRE):
                x_t = xin.tile([128, K], f32, tag="xin", name=f"xpre{mi}")
                nc.sync.dma_start(out=x_t, in_=x_ext[ts(mi, 128), :])
                x_pre[mi] = x_t
            wtot = small.tile([128, 1], f32, tag="small")
            nc.vector.tensor_reduce(out=wtot, in_=wsums, axis=AxX, op=Alu.add)
            ptot = pt.tile([1, 1], f32, tag="pt")
            nc.tensor.matmul(ptot, lhsT=ones_col, rhs=wtot, start=True, stop=True)
            # meanc = max(mean|W|, 1e-5); sw = 1/meanc; q = meanc/127
            s_meanc = small.tile([1, 1], f32, tag="s1")
            nc.vector.tensor_scalar(
                out=s_meanc,
                in0=ptot,
                scalar1=1.0 / (K * N),
                scalar2=1e-5,
                op0=Alu.mult,
                op1=Alu.max,
            )
            s_sw = small.tile([1, 1], f32, tag="s1")
            nc.vector.reciprocal(out=s_sw, in_=s_meanc)
            s_q = small.tile([1, 1], f32, tag="s1")
            nc.vector.tensor_scalar_mul(out=s_q, in0=s_meanc, scalar1=1.0 / 127.0)
            # broadcast scalars to all 128 partitions via PE outer product
            pb = pt.tile([128, 1], f32, tag="pt")
            nc.tensor.matmul(pb, lhsT=ones_row, rhs=s_sw, start=True, stop=True)
            sw_b = singles.tile([128, 1], f32)
            nc.vector.tensor_copy(out=sw_b, in_=pb)
            pb2 = pt.tile([128, 1], f32, tag="pt")
            nc.tensor.matmul(pb2, lhsT=ones_row, rhs=s_q, start=True, stop=True)
            q_b = singles.tile([128, 1], f32)
            nc.vector.tensor_copy(out=q_b, in_=pb2)

            # ---- W pass 2: qwT = clip(round(wT*sw), -1, 1) as fp8
            # Two passes, one per engine: DVE computes u = w*sw + MAGIC
            # in-place (the fp32 add rounds to the nearest integer, RNE);
            # ACT then emits Sign(u - MAGIC) straight to fp8 -- for integer
            # n, sign(n) == clip(n, -1, 1).
            for kt in range(KT):
                wt_t = wcache_tiles.get(kt) or wreread_tiles.get(kt)
                nc.vector.tensor_scalar(
                    out=wt_t, in0=wt_t, scalar1=sw_b, scalar2=MAGIC,
                    op0=Alu.mult, op1=Alu.add,
                )
                nc.scalar.activation(
                    out=qwT[:, kt, :], in_=wt_t, func=Act.Sign, bias=negmagic
                )

            # ---- main loop over row tiles
            def mm(acc, qxT, kt, nq):
                nc.tensor.matmul(
                    acc, lhsT=qxT[:, kt, :], rhs=qwT[:, kt, ts(nq, 512)],
                    start=(kt == 0), stop=(kt == KT - 1),
                    skip_group_check=True,
                )

            def finish(mi, accs, amc):
                cs = small.tile([128, 1], f32, tag="small")
                nc.vector.tensor_mul(cs, amc, q_b)  # (amax/127)*meanc
                o_t = outp.tile([128, N], f16, tag="outp", name=f"o{mi}")
                for nq in range(NQ):
                    nc.scalar.activation(
                        out=o_t[:, ts(nq, 512)], in_=accs[nq],
                        func=Act.Copy, scale=cs,
                    )
                nc.scalar.dma_start(out=out_ext[ts(mi, 128), :], in_=o_t)

            if MT >= 2:
                # interleave the first two row tiles across kt so each
                # arriving qwT k-tile unlocks 7 matmuls during the W-prep ramp
                qxT0, amc0 = xq0
                qxT1, amc1 = xq1
                accs0 = [
                    pacc.tile([128, 512], f32, tag="acc", name=f"acc_0_{i}")
                    for i in range(NQ)
                ]
                accs1 = [
                    pacc.tile([128, 512], f32, tag="acc", name=f"acc_1_{i}")
                    for i in range(NQ - 1)
                ]
                for kt in range(KT):
                    for nq in range(NQ):
                        mm(accs0[nq], qxT0, kt, nq)
                    for nq in range(NQ - 1):
                        mm(accs1[nq], qxT1, kt, nq)
                finish(0, accs0, amc0)
                acc1_last = pacc.tile(
                    [128, 512], f32, tag="acc", name="acc_1_3"
                )
                for kt in range(KT):
                    mm(acc1_last, qxT1, kt, NQ - 1)
                finish(1, accs1 + [acc1_last], amc1)
                start_mi = 2
            else:
                start_mi = 0

            for mi in range(start_mi, MT):
                qxT, amc = x_quant(mi)
                accs = [
                    pacc.tile([128, 512], f32, tag="acc", name=f"acc_{mi}_{i}")
                    for i in range(NQ)
                ]
                for kt in range(KT):
                    for nq in range(NQ):
                        mm(accs[nq], qxT, kt, nq)
                finish(mi, accs, amc)

    nc.compile()
    return nc


_NC_CACHE = {}


def _get_nc(rows_per_core):
    if rows_per_core not in _NC_CACHE:
        _NC_CACHE[rows_per_core] = build(rows_per_core)
    return _NC_CACHE[rows_per_core]


def run(x, weight, **spmd_kwargs):
    x = np.ascontiguousarray(np.asarray(x, dtype=np.float32))
    weight = np.asarray(weight, dtype=np.float32)
    b, s, k = x.shape
    rows = b * s
    rpc = rows // N_CORES
    xr = x.reshape(rows, k)
    wt = np.ascontiguousarray(weight.T)
    nc = _get_nc(rpc)
    in_maps = [
        {"x": xr[i * rpc : (i + 1) * rpc], "wt": wt} for i in range(N_CORES)
    ]
    res = run_bass_kernel_spmd(
        nc, in_maps, core_ids=list(range(N_CORES)), **spmd_kwargs
    )
    out = np.concatenate(
        [res.results[i]["out"] for i in range(N_CORES)], axis=0
    )
    return out.reshape(b, s, N), res


def kernel(x, weight):
    out, _ = run(x, weight)
    return out


# revision 25
# speedup vs baseline: 1.0441x; 1.0390x over previous
"""BitLinear (int8-activation x ternary-weight) matmul on 8 TRN2 NeuronCores.

Full inputs: x [4, 4096, 2048] f32, weight [2048, 2048] f32.
Output: [4, 4096, 2048] fp16 = ((qx @ qw.T) / si / sw).astype(f16).

Strategy: data-parallel over the 16384 rows (2048 rows/core). The weight
is replicated; each core computes mean|W| on-device (first W read),
then quantizes W to ternary {-1,0,1} stored as fp8 (cached k-tiles in
SBUF avoid most of the second read). Per-row activation quantization to
int8 values held in bf16 uses the fp32 magic-number trick
(v + 1.5*2^23 rounds to the nearest integer, RNE) and a DMA-xbar
block transpose. The matmul runs bf16(lhsT=qx^T) x fp8(qw^T) on the
TensorEngine with fp32 PSUM accumulation -- exact for these integer
values -- and the dequant (acc * amax/127 * mean|W|) is fused into the
PSUM->SBUF fp16 copy on the ScalarEngine. The first two row tiles are
interleaved across k so each arriving quantized W k-tile unlocks 7
matmuls during the W-prep ramp. Host only reshapes/shards and
transposes W (layout prep, no math).
"""

import numpy as np

import concourse.bass as bass
import concourse.mybir as mybir
import concourse.tile as tile
from concourse import bacc
from concourse.bass import ts
from concourse.bass_utils import run_bass_kernel_spmd

N_CORES = 8
ROWS_TOTAL = 4 * 4096
K = 2048
N = 2048
NCACHE = 7  # W k-tiles kept in SBUF between the mean pass and quantize pass
MAGIC = 12582912.0  # 1.5*2^23: fp32 round-to-nearest-even (both signs)

f32 = mybir.dt.float32
bf16 = mybir.dt.bfloat16
f16 = mybir.dt.float16
fp8 = mybir.dt.float8e4
Alu = mybir.AluOpType
Act = mybir.ActivationFunctionType
AxX = mybir.AxisListType.X


def build(rows_per_core=ROWS_TOTAL // N_CORES):
    nc = bacc.Bacc(
        "TRN2", target_bir_lowering=False, debug=False, num_devices=N_CORES
    )
    x_ext = nc.declare_dram_parameter("x", [rows_per_core, K], f32, isOutput=False)
    wt_ext = nc.declare_dram_parameter("wt", [K, N], f32, isOutput=False)
    out_ext = nc.declare_dram_parameter(
        "out", [rows_per_core, N], f16, isOutput=True
    )

    KT = K // 128
    MT = rows_per_core // 128
    NQ = N // 512
    NPRE = min(4, MT)  # x tiles prefetched during W prep

    with tile.TileContext(nc) as tc:
        with (
            tc.tile_pool(name="xin", bufs=4) as xin,  # [128,K] f32 x loads
            tc.tile_pool(name="wld", bufs=3) as wld,  # [128,K] f32 W loads
            tc.tile_pool(name="wch", bufs=NCACHE) as wch,  # cached W tiles
            tc.tile_pool(name="scaled", bufs=2) as scaled,  # [128,K] f32 ACT out
            tc.tile_pool(name="qtmp", bufs=2) as qtmp,  # rounded f32 / qx bf16
            tc.tile_pool(name="qxt", bufs=2) as qxtp,  # [128,KT,128] bf16 x^T
            tc.tile_pool(name="outp", bufs=3) as outp,  # [128,N] f16 results
            tc.tile_pool(name="singles", bufs=1) as singles,
            tc.tile_pool(name="small", bufs=6) as small,  # [128,1] stats
            tc.tile_pool(name="pacc", bufs=7, space="PSUM") as pacc,
            tc.tile_pool(name="pt", bufs=1, space="PSUM") as pt,
        ):
            ones_col = singles.tile([128, 1], f32)
            nc.vector.memset(ones_col, 1.0)
            ones_row = singles.tile([1, 128], f32)
            nc.vector.memset(ones_row, 1.0)
            qwT = singles.tile([128, KT, N], fp8)
            negmagic = singles.tile([128, 1], f32)
            nc.vector.memset(negmagic, -MAGIC)
            wsums = singles.tile([128, KT], f32)

            def x_quant(mi):
                if mi in x_pre:
                    x_t = x_pre[mi]
                else:
                    x_t = xin.tile([128, K], f32, tag="xin", name=f"x{mi}")
                    nc.sync.dma_start(out=x_t, in_=x_ext[ts(mi, 128), :])
                amax = small.tile([128, 1], f32, tag="small")
                nc.vector.tensor_reduce(
                    out=amax, in_=x_t, axis=AxX, op=Alu.max,
                    apply_absolute_value=True,
                )
                amc = small.tile([128, 1], f32, tag="amc", name=f"amc{mi}")
                nc.vector.tensor_scalar_max(out=amc, in0=amax, scalar1=1e-5)
                rec = small.tile([128, 1], f32, tag="small")
                nc.vector.reciprocal(out=rec, in_=amc)
                si = small.tile([128, 1], f32, tag="small")
                nc.vector.tensor_scalar_mul(out=si, in0=rec, scalar1=127.0)
                xs = scaled.tile([128, K], f32, tag="scaled")
                nc.scalar.activation(out=xs, in_=x_t, func=Act.Copy, scale=si)
                qx = qtmp.tile([128, K], bf16, tag="qtmp")
                nc.vector.tensor_scalar(
                    out=qx, in0=xs, scalar1=MAGIC, scalar2=-MAGIC,
                    op0=Alu.add, op1=Alu.add,
                )
                qxT = qxtp.tile(
                    [128, KT, 128], bf16, tag="qxt", name=f"qxT{mi}"
                )
                nc.scalar.dma_start_transpose(out=qxT, in_=qx)
                return qxT, amc

            # ---- W pass 1: mean(|W|); cache k-tiles 0..NCACHE-1 in SBUF
            wcache_tiles = {}
            for kt in range(KT):
                if kt < NCACHE:
                    wt_t = wch.tile([128, K], f32, tag="wch", name=f"wch{kt}")
                    wcache_tiles[kt] = wt_t
                else:
                    wt_t = wld.tile([128, K], f32, tag="wld", name=f"wld{kt}")
                nc.sync.dma_start(out=wt_t, in_=wt_ext[ts(kt, 128), :])
                nc.vector.tensor_reduce(
                    out=wsums[:, kt : kt + 1],
                    in_=wt_t,
                    axis=AxX,
                    op=Alu.add,
                    apply_absolute_value=True,
                )
            # first two x tiles next in the input FIFO: needed for the ramp
            x_pre = {}
            for mi in range(min(2, MT)):
                x_t = xin.tile([128, K], f32, tag="xin", name=f"xpre{mi}")
                nc.sync.dma_start(out=x_t, in_=x_ext[ts(mi, 128), :])
                x_pre[mi] = x_t
            if MT >= 2:
                xq0 = x_quant(0)
                xq1 = x_quant(1)
            # re-read DMAs for non-cached tiles (no dependency on sw)
            wreread_tiles = {}
            for kt in range(NCACHE, KT):
                wt_t = wld.tile([128, K], f32, tag="wld", name=f"wldr{kt}")
                nc.sync.dma_start(out=wt_t, in_=wt_ext[ts(kt, 128), :])
                wreread_tiles[kt] = wt_t
            # then the next prefetched x tiles
            for mi in range(2, NPRE):
                x_t = xin.tile([128, K], f32, tag="xin", name=f"xpre{mi}")
                nc.sync.dma_start(out=x_t, in_=x_ext[ts(mi, 128), :])
                x_pre[mi] = x_t
            wtot = small.tile([128, 1], f32, tag="small")
            nc.vector.tensor_reduce(out=wtot, in_=wsums, axis=AxX, op=Alu.add)
            ptot = pt.tile([1, 1], f32, tag="pt")
            nc.tensor.matmul(ptot, lhsT=ones_col, rhs=wtot, start=True, stop=True)
            # meanc = max(mean|W|, 1e-5); sw = 1/meanc; q = meanc/127
            s_meanc = small.tile([1, 1], f32, tag="s1")
            nc.vector.tensor_scalar(
                out=s_meanc,
                in0=ptot,
                scalar1=1.0 / (K * N),
                scalar2=1e-5,
                op0=Alu.mult,
                op1=Alu.max,
            )
            s_sw = small.tile([1, 1], f32, tag="s1")
            nc.vector.reciprocal(out=s_sw, in_=s_meanc)
            s_q = small.tile([1, 1], f32, tag="s1")
            nc.vector.tensor_scalar_mul(out=s_q, in0=s_meanc, scalar1=1.0 / 127.0)
            # broadcast scalars to all 128 partitions via PE outer product
            pb = pt.tile([128, 1], f32, tag="pt")
            nc.tensor.matmul(pb, lhsT=ones_row, rhs=s_sw, start=True, stop=True)
            sw_b = singles.tile([128, 1], f32)
            nc.vector.tensor_copy(out=sw_b, in_=pb)
            pb2 = pt.tile([128, 1], f32, tag="pt")
            nc.tensor.matmul(pb2, lhsT=ones_row, rhs=s_q, start=True, stop=True)
            q_b = singles.tile([128, 1], f32)
            nc.vector.tensor_copy(out=q_b, in_=pb2)

            # ---- W pass 2: qwT = clip(round(wT*sw), -1, 1) as fp8
            # Two passes, one per engine: DVE computes u = w*sw + MAGIC
            # in-place (the fp32 add rounds to the nearest integer, RNE);
            # ACT then emits Sign(u - MAGIC) straight to fp8 -- for integer
            # n, sign(n) == clip(n, -1, 1).
            for kt in range(KT):
                wt_t = wcache_tiles.get(kt) or wreread_tiles.get(kt)
                nc.vector.tensor_scalar(
                    out=wt_t, in0=wt_t, scalar1=sw_b, scalar2=MAGIC,
                    op0=Alu.mult, op1=Alu.add,
                )
                nc.scalar.activation(
                    out=qwT[:, kt, :], in_=wt_t, func=Act.Sign, bias=negmagic
                )

            # ---- main loop over row tiles
            def mm(acc, qxT, kt, nq):
                nc.tensor.matmul(
                    acc, lhsT=qxT[:, kt, :], rhs=qwT[:, kt, ts(nq, 512)],
                    start=(kt == 0), stop=(kt == KT - 1),
                    skip_group_check=True,
                )

            def finish(mi, accs, amc):
                cs = small.tile([128, 1], f32, tag="small")
                nc.vector.tensor_mul(cs, amc, q_b)  # (amax/127)*meanc
                o_t = outp.tile([128, N], f16, tag="outp", name=f"o{mi}")
                for nq in range(NQ):
                    nc.scalar.activation(
                        out=o_t[:, ts(nq, 512)], in_=accs[nq],
                        func=Act.Copy, scale=cs,
                    )
                nc.scalar.dma_start(out=out_ext[ts(mi, 128), :], in_=o_t)

            if MT >= 2:
                # interleave the first two row tiles across kt so each
                # arriving qwT k-tile unlocks 7 matmuls during the W-prep ramp
                qxT0, amc0 = xq0
                qxT1, amc1 = xq1
                accs0 = [
                    pacc.tile([128, 512], f32, tag="acc", name=f"acc_0_{i}")
                    for i in range(NQ)
                ]
                accs1 = [
                    pacc.tile([128, 512], f32, tag="acc", name=f"acc_1_{i}")
                    for i in range(NQ - 1)
                ]
                for kt in range(KT):
                    for nq in range(NQ):
                        mm(accs0[nq], qxT0, kt, nq)
                    for nq in range(NQ - 1):
                        mm(accs1[nq], qxT1, kt, nq)
                finish(0, accs0, amc0)
                acc1_last = pacc.tile(
                    [128, 512], f32, tag="acc", name="acc_1_3"
                )
                for kt in range(KT):
                    mm(acc1_last, qxT1, kt, NQ - 1)
                finish(1, accs1 + [acc1_last], amc1)
                start_mi = 2
            else:
                start_mi = 0

            for mi in range(start_mi, MT):
                qxT, amc = x_quant(mi)
                accs = [
                    pacc.tile([128, 512], f32, tag="acc", name=f"acc_{mi}_{i}")
                    for i in range(NQ)
                ]
                if mi == MT - 1:
                    # nq-inner: each output chunk completes as soon as its
                    # 16 accumulations are done, so the dequant + store
                    # overlap the remaining matmuls (shorter kernel tail)
                    for nq in range(NQ):
                        for kt in range(KT):
                            mm(accs[nq], qxT, kt, nq)
                else:
                    for kt in range(KT):
                        for nq in range(NQ):
                            mm(accs[nq], qxT, kt, nq)
                finish(mi, accs, amc)

    nc.compile()
    return nc


_NC_CACHE = {}


def _get_nc(rows_per_core):
    if rows_per_core not in _NC_CACHE:
        _NC_CACHE[rows_per_core] = build(rows_per_core)
    return _NC_CACHE[rows_per_core]


def run(x, weight, **spmd_kwargs):
    x = np.ascontiguousarray(np.asarray(x, dtype=np.float32))
    weight = np.asarray(weight, dtype=np.float32)
    b, s, k = x.shape
    rows = b * s
    rpc = rows // N_CORES
    xr = x.reshape(rows, k)
    wt = np.ascontiguousarray(weight.T)
    nc = _get_nc(rpc)
    in_maps = [
        {"x": xr[i * rpc : (i + 1) * rpc], "wt": wt} for i in range(N_CORES)
    ]
    res = run_bass_kernel_spmd(
        nc, in_maps, core_ids=list(range(N_CORES)), **spmd_kwargs
    )
    out = np.concatenate(
        [res.results[i]["out"] for i in range(N_CORES)], axis=0
    )
    return out.reshape(b, s, N), res


def kernel(x, weight):
    out, _ = run(x, weight)
    return out
